# revision 1
# baseline (speedup 1.0000x reference)
"""Trainium2 Bass kernel: cached multi-head self-attention decoder block.

Per-core (batch-parallel, B=8 -> 8 cores) computation for batch b:
  q  = x @ Wq + bq        (kept transposed: qT [NS, T], pre-scaled by HD^-0.5)
  kn = x @ Wk             (kept transposed: knT [NS, T])
  vn = x @ Wv + bv        (natural [T, NS])
  k  = concat(cache_kT, knT)  [NS, S+T]   (head-dim on partitions)
  scoresT[h] = kh^T-slices x qh  -> [S+T, T] per head (s on partitions)
  probsT = exp(scoresT)  (no max-subtraction needed: |scores| <~ 8)
  o[h]   = vh'^T @ probsT  accumulated over s-chunks, where vh' = [vh | 1]
           -> rows 0..63 = unnormalized o^T, row 64 = softmax denominator
  wvT    = o * (1/denom broadcast)
  outT   = Wo^T @ wvT + bo
Host side transposes xT/kT inputs and outT/keyT outputs.
"""

import numpy as np
from contextlib import ExitStack

import concourse.bass as bass
import concourse.tile as tile
from concourse import bacc, mybir
from concourse.bass_utils import run_bass_kernel_spmd

F32 = mybir.dt.float32
F32R = mybir.dt.float32r
ALU = mybir.AluOpType
ACTF = mybir.ActivationFunctionType

B, T, S, NS, NH, HD = 8, 512, 2048, 1024, 16, 64
ST = S + T            # 2560
NC_CHUNKS = NS // 128  # 8 ns chunks (= head pairs)
SCN = ST // 128       # 20 s chunks (16 cache + 4 new)
SCALE2 = float(HD ** -0.5)  # fold both q and k scales into q

LAST_EXEC_NS = None
LAST_RESULTS = None


def _emit(ctx, tc, D):
    nc = tc.nc

    # ---------------- constants / persistent tiles ----------------
    const = ctx.enter_context(tc.tile_pool(name="const", bufs=1))
    bqs_t = const.tile([128, 8], F32, name="bqs_t")
    nc.sync.dma_start(bqs_t[:], D["bqs"][:, :])
    bop_t = const.tile([128, 8], F32, name="bop_t")
    nc.sync.dma_start(bop_t[:], D["bop"][:, :])
    bv_t = const.tile([1, NS], F32R, name="bv_t")
    nc.gpsimd.dma_start(bv_t[:], D["bv"][:, :])
    ones1 = const.tile([1, 128], F32R, name="ones1")
    nc.gpsimd.dma_start(ones1[:], D["ones"].ap()[0:1, 0:128])
    onesp_src = D["ones"].ap()[:, 128:136]

    xT_t = const.tile([128, 4096], F32R, name="xT_t")  # k-chunk-major cols
    nc.gpsimd.dma_start(
        xT_t[:].rearrange("p (k t) -> p k t", k=8),
        D["xT"].ap().rearrange("(k p) t -> p k t", p=128),
    )

    pers = ctx.enter_context(tc.tile_pool(name="pers", bufs=1))
    qT_t = pers.tile([128, 4096], F32R, name="qT_t")   # (x@Wq+bq)*s2, m-chunk-major
    knT_t = pers.tile([128, 4096], F32R, name="knT_t")  # (x@Wk)^T, matmul copy
    knTf_t = pers.tile([128, 4096], F32, name="knTf_t")  # (x@Wk)^T, exact f32 for keyT
    vnew_t = pers.tile([128, 4096], F32, name="vnew_t")  # x@Wv+bv natural, tc-major
    outT_t = pers.tile([128, 4096], F32, name="outT_t")

    wv_pool = ctx.enter_context(tc.tile_pool(name="wv", bufs=1))
    wv_tiles = [wv_pool.tile([128, 512], F32R, name=f"wv_{c}", tag=f"wv{c}")
                for c in range(NC_CHUNKS)]

    wo_pool = ctx.enter_context(tc.tile_pool(name="wo", bufs=1))
    wo_tiles = [wo_pool.tile([128, 1024], F32R, name=f"wo_{k}", tag=f"wo{k}")
                for k in range(8)]

    # ---------------- phase P: projections ----------------
    with ExitStack() as pctx:
        wpool = pctx.enter_context(tc.tile_pool(name="wstream", bufs=3))
        ppool = pctx.enter_context(tc.tile_pool(name="pproj", bufs=1, space="PSUM"))

        def proj_T(w_dram, out_tile, bias_col, scale, extra_f32=None):
            """out_tile[:, m*512:(m+1)*512] = ((x @ W)^T chunk m)*scale + bias."""
            pts = [ppool.tile([128, 512], F32, name=f"pp{m}", tag=f"pp{m}")
                   for m in range(8)]
            for k in range(8):
                wt = wpool.tile([128, 1024], F32R, name="wt", tag="w")
                nc.gpsimd.dma_start(wt[:], w_dram[k * 128:(k + 1) * 128, :])
                for m in range(8):
                    nc.tensor.matmul(
                        pts[m][:],
                        lhsT=wt[:, m * 128:(m + 1) * 128],
                        rhs=xT_t[:, k * 512:(k + 1) * 512],
                        start=(k == 0), stop=(k == 7),
                    )
            for m in range(8):
                dst = out_tile[:, m * 512:(m + 1) * 512]
                if bias_col is not None:
                    nc.vector.tensor_scalar(
                        dst, pts[m][:], scale, bias_col[:, m:m + 1],
                        ALU.mult, ALU.add)
                else:
                    nc.vector.tensor_copy(dst, pts[m][:])
                if extra_f32 is not None:
                    nc.vector.tensor_copy(
                        extra_f32[:, m * 512:(m + 1) * 512], pts[m][:])

        proj_T(D["Wq"].ap(), qT_t, bqs_t, SCALE2)
        proj_T(D["Wk"].ap(), knT_t, None, 1.0, extra_f32=knTf_t)
        # keyT output straight from knT
        nc.sync.dma_start(
            D["keyT"].ap().rearrange("(m p) t -> p m t", p=128),
            knTf_t[:].rearrange("p (m t) -> p m t", m=8),
        )

        # v natural: out rows t (tc chunks), cols o (2 halves); contract over ns
        vps = [ppool.tile([128, 512], F32, name=f"pv{g}", tag=f"pp{g}")
               for g in range(8)]
        for k in range(8):
            wt = wpool.tile([128, 1024], F32R, name="wt", tag="w")
            nc.gpsimd.dma_start(wt[:], D["Wv"].ap()[k * 128:(k + 1) * 128, :])
            for tc_i in range(4):
                for oh in range(2):
                    nc.tensor.matmul(
                        vps[tc_i * 2 + oh][:],
                        lhsT=xT_t[:, k * 512 + tc_i * 128:
                                  k * 512 + (tc_i + 1) * 128],
                        rhs=wt[:, oh * 512:(oh + 1) * 512],
                        start=(k == 0), stop=False,
                    )
        for tc_i in range(4):
            for oh in range(2):
                # bias row via K=1 matmul: + ones^T @ bv_slice
                nc.tensor.matmul(
                    vps[tc_i * 2 + oh][:],
                    lhsT=ones1[:],
                    rhs=bv_t[0:1, oh * 512:(oh + 1) * 512],
                    start=False, stop=True,
                )
                nc.vector.tensor_copy(
                    vnew_t[:, tc_i * 1024 + oh * 512:tc_i * 1024 + (oh + 1) * 512],
                    vps[tc_i * 2 + oh][:])
        nc.sync.dma_start(
            D["value"].ap().rearrange("(tc p) o -> p tc o", p=128),
            vnew_t[:].rearrange("p (tc o) -> p tc o", tc=4),
        )

    # ---------------- phase A: attention, one head-pair per ns-chunk ----------------
    with ExitStack() as actx:
        kpool = actx.enter_context(tc.tile_pool(name="kpair", bufs=2))
        vpool = actx.enter_context(tc.tile_pool(name="vpair", bufs=2))
        probs_pool = actx.enter_context(tc.tile_pool(name="probs", bufs=3))
        spool = actx.enter_context(tc.tile_pool(name="spsum", bufs=3, space="PSUM"))
        pvpool = actx.enter_context(tc.tile_pool(name="pvpsum", bufs=1, space="PSUM"))
        rpool = actx.enter_context(tc.tile_pool(name="rtiles", bufs=1))

        for c in range(NC_CHUNKS):
            kp = kpool.tile([128, 2048], F32R, name="kp", tag="kp")
            nc.gpsimd.dma_start(kp[:], D["kTc"].ap()[c * 128:(c + 1) * 128, :])
            vp = vpool.tile([128, 2600], F32R, name="vp", tag="vp")
            nc.gpsimd.dma_start(
                vp[:, 0:2080].rearrange("p (j q) -> p j q", q=130),
                D["vaug"].ap().rearrange("(j p) q -> p j q", p=128)
                [:, :, c * 130:(c + 1) * 130],
            )
            # new-token v blocks [v_even|1|v_odd|1] at cols 2080 + tc*130
            nc.gpsimd.dma_start(
                vp[:, 2080:2600].rearrange("p (tc h q) -> p tc h q", h=2, q=65)
                [:, :, :, 64:65],
                onesp_src.rearrange("p (tc h q) -> p tc h q", tc=4, h=2))
            for tc_i in range(4):
                nc.vector.tensor_copy(
                    vp[:, 2080 + tc_i * 130:2080 + (tc_i + 1) * 130]
                    .rearrange("p (h q) -> p h q", q=65)[:, :, 0:64],
                    vnew_t[:, tc_i * 1024 + c * 128:tc_i * 1024 + (c + 1) * 128]
                    .rearrange("p (h q) -> p h q", q=64),
                )

            if c == 1:
                # prefetch Wo while attention runs
                for k in range(8):
                    nc.gpsimd.dma_start(wo_tiles[k][:],
                                        D["Wo"].ap()[k * 128:(k + 1) * 128, :])

            pve = pvpool.tile([65, 512], F32, name="pve", tag="pv_e")
            pvo = pvpool.tile([65, 512], F32, name="pvo", tag="pv_o")
            rhs_e = qT_t[0:64, c * 512:(c + 1) * 512]
            rhs_o = qT_t[64:128, c * 512:(c + 1) * 512]

            for g in range(10):
                se = spool.tile([128, 1024], F32, name="se", tag="sc")
                so = spool.tile([128, 1024], F32, name="so", tag="sc")
                for jj in range(2):
                    j = 2 * g + jj
                    if j < 16:
                        le = kp[0:64, j * 128:(j + 1) * 128]
                        lo = kp[64:128, j * 128:(j + 1) * 128]
                    else:
                        jo = c * 512 + (j - 16) * 128
                        le = knT_t[0:64, jo:jo + 128]
                        lo = knT_t[64:128, jo:jo + 128]
                    nc.tensor.matmul(se[:, jj * 512:(jj + 1) * 512],
                                     lhsT=le, rhs=rhs_e,
                                     start=True, stop=True)
                    nc.tensor.matmul(so[:, jj * 512:(jj + 1) * 512],
                                     lhsT=lo, rhs=rhs_o,
                                     start=True, stop=True)
                pe_t = probs_pool.tile([128, 1024], F32R, name="pe_t", tag="pr")
                nc.scalar.activation(pe_t[:], se[:], ACTF.Exp)
                po_t = probs_pool.tile([128, 1024], F32R, name="po_t", tag="pr")
                nc.scalar.activation(po_t[:], so[:], ACTF.Exp)
                for jj in range(2):
                    j = 2 * g + jj
                    nc.tensor.matmul(
                        pve[:],
                        lhsT=vp[:, j * 130:j * 130 + 65],
                        rhs=pe_t[:, jj * 512:(jj + 1) * 512],
                        start=(j == 0), stop=(j == SCN - 1))
                    nc.tensor.matmul(
                        pvo[:],
                        lhsT=vp[:, j * 130 + 65:(j + 1) * 130],
                        rhs=po_t[:, jj * 512:(jj + 1) * 512],
                        start=(j == 0), stop=(j == SCN - 1))

            # normalize: rows 0..63 = o^T unnormalized, row 64 = denom
            rde = rpool.tile([65, 512], F32, name="rde", tag="rd")
            nc.vector.reciprocal(rde[64:65, :], pve[64:65, :])
            rd0e = rpool.tile([1, 512], F32, name="rd0e", tag="rd0e")
            nc.sync.dma_start(rd0e[:], rde[64:65, :])  # move to physical part 0
            rbe = rpool.tile([64, 512], F32, name="rbe", tag="rb")
            nc.gpsimd.partition_broadcast(rbe[:], rd0e[:])
            nc.vector.tensor_mul(wv_tiles[c][0:64, :], pve[0:64, :], rbe[:])

            rdo = rpool.tile([65, 512], F32, name="rdo", tag="rd")
            nc.vector.reciprocal(rdo[64:65, :], pvo[64:65, :])
            rd0o = rpool.tile([1, 512], F32, name="rd0o", tag="rd0o")
            nc.sync.dma_start(rd0o[:], rdo[64:65, :])
            rbo = rpool.tile([64, 512], F32, name="rbo", tag="rb")
            nc.gpsimd.partition_broadcast(rbo[:], rd0o[:])
            tmo = rpool.tile([64, 512], F32R, name="tmo", tag="tm")
            nc.vector.tensor_mul(tmo[:], pvo[0:64, :], rbo[:])
            # partition shift 0..63 -> 64..127 via SBUF->SBUF DMA
            nc.sync.dma_start(wv_tiles[c][64:128, :], tmo[:])
            if c == 0:
                nc.gpsimd.dma_start(D["dbg"].ap(), wv_tiles[0][:])

    # ---------------- phase O: output projection ----------------
    with ExitStack() as octx:
        opool = octx.enter_context(tc.tile_pool(name="opsum", bufs=2, space="PSUM"))
        for m in range(8):
            po = opool.tile([128, 512], F32, name="po", tag="po")
            for c in range(8):
                nc.tensor.matmul(
                    po[:],
                    lhsT=wo_tiles[c][:, m * 128:(m + 1) * 128],
                    rhs=wv_tiles[c][:],
                    start=(c == 0), stop=(c == 7))
            nc.vector.tensor_scalar(
                outT_t[:, m * 512:(m + 1) * 512], po[:], 1.0, bop_t[:, m:m + 1],
                ALU.mult, ALU.add)
        nc.sync.dma_start(
            D["outT"].ap().rearrange("(m p) t -> p m t", p=128),
            outT_t[:].rearrange("p (m t) -> p m t", m=8),
        )


def build():
    nc = bacc.Bacc("TRN2", target_bir_lowering=False, debug=False)
    D = {}
    D["xT"] = nc.dram_tensor("xT", [NS, T], F32, kind="ExternalInput")
    D["kTc"] = nc.dram_tensor("kTc", [NS, S], F32, kind="ExternalInput")
    D["vaug"] = nc.dram_tensor("vaug", [S, NH * 65], F32, kind="ExternalInput")
    for w in ("Wq", "Wk", "Wv", "Wo"):
        D[w] = nc.dram_tensor(w, [NS, NS], F32, kind="ExternalInput")
    D["bqs"] = nc.dram_tensor("bqs", [128, 8], F32, kind="ExternalInput")
    D["bop"] = nc.dram_tensor("bop", [128, 8], F32, kind="ExternalInput")
    D["bv"] = nc.dram_tensor("bv", [1, NS], F32, kind="ExternalInput")
    D["ones"] = nc.dram_tensor("ones", [128, 136], F32, kind="ExternalInput")
    D["outT"] = nc.dram_tensor("outT", [NS, T], F32, kind="ExternalOutput")
    D["keyT"] = nc.dram_tensor("keyT", [NS, T], F32, kind="ExternalOutput")
    D["value"] = nc.dram_tensor("value", [T, NS], F32, kind="ExternalOutput")
    D["dbg"] = nc.dram_tensor("dbg", [128, 512], F32, kind="ExternalOutput")

    with tile.TileContext(nc) as tc:
        with ExitStack() as ctx:
            _emit(ctx, tc, D)
    nc.compile()
    return nc


_NC_CACHE = None


def _get_nc():
    global _NC_CACHE
    if _NC_CACHE is None:
        _NC_CACHE = build()
    return _NC_CACHE


def prep_core_inputs(b, x, kv_cache, Wq, bq, Wk, Wv, bv, Wo, bo):
    xT = np.ascontiguousarray(x[b].T)                      # [NS, T]
    kTc = np.ascontiguousarray(kv_cache[b, 0, 0].T)        # [NS, S]
    vc = kv_cache[b, 0, 1]                                 # [S, NS]
    vaug = np.empty((S, NH * 65), np.float32)
    va = vaug.reshape(S, NH, 65)
    va[:, :, 0:64] = vc.reshape(S, NH, 64)
    va[:, :, 64] = 1.0
    return {
        "xT": xT, "kTc": kTc, "vaug": vaug,
        "Wq": Wq, "Wk": Wk, "Wv": Wv, "Wo": Wo,
        "bqs": np.ascontiguousarray((bq * SCALE2).reshape(8, 128).T),
        "bop": np.ascontiguousarray(bo.reshape(8, 128).T),
        "bv": np.ascontiguousarray(bv[None, :]),
        "ones": np.ones((128, 136), np.float32),
    }


def kernel(x, kv_cache, offset=0, Wq=None, bq=None, Wk=None, Wv=None, bv=None,
           Wo=None, bo=None, trace=False):
    global LAST_EXEC_NS, LAST_RESULTS
    x = np.asarray(x, np.float32)
    kv_cache = np.asarray(kv_cache, np.float32)
    args = [np.asarray(a, np.float32) for a in (Wq, bq, Wk, Wv, bv, Wo, bo)]
    in_maps = [prep_core_inputs(b, x, kv_cache, *args) for b in range(B)]
    nc = _get_nc()
    res = run_bass_kernel_spmd(nc, in_maps, core_ids=list(range(B)), trace=trace)
    LAST_EXEC_NS = res.exec_time_ns
    LAST_RESULTS = res
    out = np.stack([res.results[b]["outT"].T for b in range(B)])
    key = np.stack([res.results[b]["keyT"].T for b in range(B)])
    value = np.stack([res.results[b]["value"] for b in range(B)])
    return (np.ascontiguousarray(out), np.ascontiguousarray(key),
            np.ascontiguousarray(value))



# revision 8
# speedup vs baseline: 1.3775x; 1.3775x over previous
"""Trainium2 Bass kernel: cached multi-head self-attention decoder block.

Per-core (batch-parallel, B=8 -> 8 cores) computation for batch b, fp16
matmul operands (fp16 streams the PE moving operand at 1 elem/cycle vs
2 cyc/elem for fp32), f32 PSUM accumulation, f32 outputs:

  q  = x @ Wq + bq        (qT [NS, T] fp16, pre-scaled by HD^-0.5)
  kn = x @ Wk             (knT fp16 for scores + f32 copy for keyT out)
  vn = x @ Wv + bv        (vnew f32 [T, NS] for value out + fp16 vp blocks)
  scoresT[h] = kh^T-slices x qh -> [S+T, T] per head (s on partitions)
  probsT = exp(scoresT - 8)  (fp16; bias keeps probs <= 1 in fp16 range)
  o[h]   = [vh | 1]^T @ probsT accumulated over s-chunks in PSUM
           rows 0..63 = unnormalized o^T, row 64 = softmax denominator
  wvT    = o * (1/denom broadcast via K=1 matmul ones x recip row)
  outT   = Wo^T @ wvT + bo

The attention phase is bound by the ScalarE exp stream (~178us); all
projection matmuls are interleaved into it as filler PE work via a task
queue, and the per-g wv matmuls are software-pipelined one stage behind
the score matmuls so the PE never stalls on the ACT->probs dependency.
"""

import numpy as np
from collections import deque
from contextlib import ExitStack

import concourse.bass as bass
import concourse.tile as tile
from concourse import bacc, mybir
from concourse.bass_utils import run_bass_kernel_spmd

F32 = mybir.dt.float32
F16 = mybir.dt.float16
ALU = mybir.AluOpType
ACTF = mybir.ActivationFunctionType

B, T, S, NS, NH, HD = 8, 512, 2048, 1024, 16, 64
ST = S + T             # 2560
NC_CHUNKS = NS // 128  # 8 head pairs
SCN = ST // 128        # 20 s chunks (16 cache + 4 new)
SCALE2 = float(HD ** -0.5)  # fold both q and k scales into q
EXP_BIAS = -8.0        # probs = exp(scores - 8) <= ~1 : fp16-safe

LAST_EXEC_NS = None
LAST_RESULTS = None


def _emit(ctx, tc, D):
    nc = tc.nc

    # ---------------- pools ----------------
    const = ctx.enter_context(tc.tile_pool(name="const", bufs=1))
    wpool = ctx.enter_context(tc.tile_pool(name="weights", bufs=1))
    pers = ctx.enter_context(tc.tile_pool(name="pers", bufs=1))
    wvp = ctx.enter_context(tc.tile_pool(name="wvp", bufs=1))
    kpool = ctx.enter_context(tc.tile_pool(name="kpair", bufs=2))
    vpool = ctx.enter_context(tc.tile_pool(name="vpair", bufs=2))
    probs = ctx.enter_context(tc.tile_pool(name="probs", bufs=4))
    rpool = ctx.enter_context(tc.tile_pool(name="rtiles", bufs=2))
    spool = ctx.enter_context(tc.tile_pool(name="spsum", bufs=2, space="PSUM"))
    pvpool = ctx.enter_context(tc.tile_pool(name="pvpsum", bufs=1, space="PSUM"))
    pmisc = ctx.enter_context(tc.tile_pool(name="pmisc", bufs=2, space="PSUM"))

    # ---------------- constants ----------------
    bqs_t = const.tile([128, 8], F32, name="bqs_t")
    nc.gpsimd.dma_start(bqs_t[:], D["bqs"][:, :])
    bop_t = const.tile([128, 8], F32, name="bop_t")
    nc.gpsimd.dma_start(bop_t[:], D["bop"][:, :])
    bv_t = const.tile([1, NS], F16, name="bv_t")
    nc.gpsimd.dma_start(bv_t[:], D["bv"][:, :])
    ones1 = const.tile([1, 128], F16, name="ones1")
    nc.gpsimd.dma_start(ones1[:], D["ones"].ap()[0:1, 0:128])
    ones64 = const.tile([128, 64], F16, name="ones64")
    nc.gpsimd.dma_start(ones64[:], D["ones"].ap()[:, 0:64])

    xT_t = const.tile([128, 4096], F16, name="xT_t")  # k-chunk-major cols
    nc.gpsimd.dma_start(
        xT_t[:].rearrange("p (k t) -> p k t", k=8),
        D["xT"].ap().rearrange("(k p) t -> p k t", p=128),
    )
    ebias_t = const.tile([128, 1], F32, name="ebias_t")
    nc.gpsimd.memset(ebias_t[:], EXP_BIAS)

    # ---------------- weights: full fp16 preload on sync queue ----------------
    # DMA order = need order: Wq (q proj prologue), Wv (v-proj from c=0),
    # Wk (k-proj m=0 by c=0 g6), Wo (tail).
    wq_t = [wpool.tile([128, 1024], F16, name=f"wq{k}", tag=f"wq{k}")
            for k in range(8)]
    wv_t = [wpool.tile([128, 1024], F16, name=f"wvw{k}", tag=f"wvw{k}")
            for k in range(8)]
    wk_t = [wpool.tile([128, 1024], F16, name=f"wk{k}", tag=f"wk{k}")
            for k in range(8)]
    wo_t = [wpool.tile([128, 1024], F16, name=f"wo{k}", tag=f"wo{k}")
            for k in range(8)]
    for k in range(8):
        nc.sync.dma_start(wq_t[k][:], D["Wq"].ap()[k * 128:(k + 1) * 128, :])
    for k in range(8):
        nc.sync.dma_start(wv_t[k][:], D["Wv"].ap()[k * 128:(k + 1) * 128, :])
    for k in range(8):
        nc.sync.dma_start(wk_t[k][:], D["Wk"].ap()[k * 128:(k + 1) * 128, :])
    for k in range(8):
        nc.sync.dma_start(wo_t[k][:], D["Wo"].ap()[k * 128:(k + 1) * 128, :])

    # ---------------- persistent activations ----------------
    qT_t = pers.tile([128, 4096], F16, name="qT_t")    # m-chunk-major
    knT_t = pers.tile([128, 4096], F16, name="knT_t")  # m-chunk-major
    knTf_t = pers.tile([128, 4096], F32, name="knTf_t")  # exact f32 for keyT
    vnew_t = pers.tile([128, 4096], F32, name="vnew_t")  # tc-major [t, ns]
    outT_t = pers.tile([128, 4096], F32, name="outT_t")
    wv_tiles = [wvp.tile([128, 512], F16, name=f"wv_{c}", tag=f"wv{c}")
                for c in range(NC_CHUNKS)]

    # ---------------- projection task emitters ----------------
    def q_chunk_tasks(m):
        st = {}
        out = []
        for k in range(8):
            def tk(k=k, m=m):
                if k == 0:
                    st["p"] = pmisc.tile([128, 512], F32, name=f"pq{m}", tag="pp")
                nc.tensor.matmul(
                    st["p"][:], lhsT=wq_t[k][:, m * 128:(m + 1) * 128],
                    rhs=xT_t[:, k * 512:(k + 1) * 512],
                    start=(k == 0), stop=(k == 7))
            out.append(tk)

        def tf(m=m):
            nc.vector.tensor_scalar(
                qT_t[:, m * 512:(m + 1) * 512], st["p"][:], SCALE2,
                bqs_t[:, m:m + 1], ALU.mult, ALU.add)
        out.append(tf)
        return out

    def k_chunk_tasks(m):
        st = {}
        out = []
        for k in range(8):
            def tk(k=k, m=m):
                if k == 0:
                    st["p"] = pmisc.tile([128, 512], F32, name=f"pk{m}", tag="pp")
                nc.tensor.matmul(
                    st["p"][:], lhsT=wk_t[k][:, m * 128:(m + 1) * 128],
                    rhs=xT_t[:, k * 512:(k + 1) * 512],
                    start=(k == 0), stop=(k == 7))
            out.append(tk)

        def tf(m=m):
            nc.vector.tensor_copy(knT_t[:, m * 512:(m + 1) * 512], st["p"][:])
            nc.vector.tensor_copy(knTf_t[:, m * 512:(m + 1) * 512], st["p"][:])
        out.append(tf)
        return out

    def v_chunk_tasks(tc_i, oh):
        st = {}
        out = []
        for k in range(8):
            def tk(k=k, tc_i=tc_i, oh=oh):
                if k == 0:
                    st["p"] = pmisc.tile([128, 512], F32,
                                         name=f"pv{tc_i}{oh}", tag="pp")
                nc.tensor.matmul(
                    st["p"][:],
                    lhsT=xT_t[:, k * 512 + tc_i * 128:k * 512 + (tc_i + 1) * 128],
                    rhs=wv_t[k][:, oh * 512:(oh + 1) * 512],
                    start=(k == 0), stop=False)
            out.append(tk)

        def tb(tc_i=tc_i, oh=oh):
            nc.tensor.matmul(
                st["p"][:], lhsT=ones1[:],
                rhs=bv_t[0:1, oh * 512:(oh + 1) * 512],
                start=False, stop=True)
            nc.vector.tensor_copy(
                vnew_t[:, tc_i * 1024 + oh * 512:tc_i * 1024 + (oh + 1) * 512],
                st["p"][:])
        out.append(tb)
        return out

    def vp_new_tasks(c, vp_t):
        out = []
        for tc_i in range(4):
            def tcpy(tc_i=tc_i, c=c, vp_t=vp_t):
                nc.vector.tensor_copy(
                    vp_t[:, 2080 + tc_i * 130:2080 + (tc_i + 1) * 130]
                    .rearrange("p (h q) -> p h q", q=65)[:, :, 0:64],
                    vnew_t[:, tc_i * 1024 + c * 128:tc_i * 1024 + (c + 1) * 128]
                    .rearrange("p (h q) -> p h q", q=64))
            out.append(tcpy)
        return out

    # ---------------- attention primitives ----------------
    def fetch_kv(c):
        kp = kpool.tile([128, 2048], F16, name="kp", tag="kp")
        nc.gpsimd.dma_start(kp[:], D["kTc"].ap()[c * 128:(c + 1) * 128, :])
        vp = vpool.tile([128, 2600], F16, name="vp", tag="vp")
        nc.gpsimd.dma_start(vp[:, 0:2080],
                            D["vaug"].ap()[:, c * 2080:(c + 1) * 2080])
        nc.gpsimd.memset(
            vp[:, 2080:2600].rearrange("p (tc h q) -> p tc h q", h=2, q=65)
            [:, :, :, 64:65], 1.0)
        return kp, vp

    def emit_scores(c, g, kp_t):
        se = spool.tile([128, 1024], F32, name="se", tag="sc")
        so = spool.tile([128, 1024], F32, name="so", tag="sc")
        rhs_e = qT_t[0:64, c * 512:(c + 1) * 512]
        rhs_o = qT_t[64:128, c * 512:(c + 1) * 512]
        for jj in range(2):
            j = 2 * g + jj
            if j < 16:
                le = kp_t[0:64, j * 128:(j + 1) * 128]
                lo = kp_t[64:128, j * 128:(j + 1) * 128]
            else:
                jo = c * 512 + (j - 16) * 128
                le = knT_t[0:64, jo:jo + 128]
                lo = knT_t[64:128, jo:jo + 128]
            nc.tensor.matmul(se[:, jj * 512:(jj + 1) * 512], lhsT=le,
                             rhs=rhs_e, start=True, stop=True)
            nc.tensor.matmul(so[:, jj * 512:(jj + 1) * 512], lhsT=lo,
                             rhs=rhs_o, start=True, stop=True)
        pe_t = probs.tile([128, 1024], F16, name="pe_t", tag="pr")
        nc.scalar.activation(pe_t[:], se[:], ACTF.Exp, bias=ebias_t[:])
        po_t = probs.tile([128, 1024], F16, name="po_t", tag="pr")
        nc.scalar.activation(po_t[:], so[:], ACTF.Exp, bias=ebias_t[:])
        return pe_t, po_t

    pv_cur = {}

    def emit_wv(g, vp_t, pe_t, po_t):
        if g == 0:
            pv_cur["e"] = pvpool.tile([65, 512], F32, name="pve", tag="pv_e")
            pv_cur["o"] = pvpool.tile([65, 512], F32, name="pvo", tag="pv_o")
        pve, pvo = pv_cur["e"], pv_cur["o"]
        for jj in range(2):
            j = 2 * g + jj
            nc.tensor.matmul(pve[:], lhsT=vp_t[:, j * 130:j * 130 + 65],
                             rhs=pe_t[:, jj * 512:(jj + 1) * 512],
                             start=(j == 0), stop=(j == SCN - 1))
            nc.tensor.matmul(pvo[:], lhsT=vp_t[:, j * 130 + 65:(j + 1) * 130],
                             rhs=po_t[:, jj * 512:(jj + 1) * 512],
                             start=(j == 0), stop=(j == SCN - 1))

    def emit_norm(c):
        pve, pvo = pv_cur.pop("e"), pv_cur.pop("o")
        with nc.allow_low_precision(reason="1/denom scale in fp16"):
            rde = rpool.tile([65, 512], F16, name="rde", tag="rd")
            nc.vector.reciprocal(rde[64:65, :], pve[64:65, :])
            rbe = pmisc.tile([128, 512], F32, name="rbe", tag="pp")
            nc.tensor.matmul(rbe[0:64, :], lhsT=ones64[64:65, 0:64],
                             rhs=rde[64:65, :], start=True, stop=True)
            rbse = rpool.tile([64, 512], F32, name="rbse", tag="rb")
            nc.vector.tensor_copy(rbse[:], rbe[0:64, :])
            nc.vector.tensor_mul(wv_tiles[c][0:64, :], pve[0:64, :],
                                 rbse[:])
            rdo = rpool.tile([65, 512], F16, name="rdo", tag="rd")
            nc.vector.reciprocal(rdo[64:65, :], pvo[64:65, :])
            rbo = pmisc.tile([128, 512], F32, name="rbo", tag="pp")
            nc.tensor.matmul(rbo[0:64, :], lhsT=ones64[64:65, 0:64],
                             rhs=rdo[64:65, :], start=True, stop=True)
            rbso = rpool.tile([64, 512], F32, name="rbso", tag="rb")
            nc.vector.tensor_copy(rbso[:], rbo[0:64, :])
            tmo = rpool.tile([64, 512], F16, name="tmo", tag="tm")
            nc.vector.tensor_mul(tmo[:], pvo[0:64, :], rbso[:])
        # partition shift 0..63 -> 64..127 via SBUF->SBUF DMA
        nc.sync.dma_start(wv_tiles[c][64:128, :], tmo[:])

    # ---------------- prologue ----------------
    kp_cur, vp_cur_t = fetch_kv(0)
    for t in q_chunk_tasks(0):
        t()
    for t in q_chunk_tasks(1):
        t()

    # ---------------- main attention loop with interleaved projections ----
    def build_tasks(c):
        tk = []
        if c == 0:
            tk += q_chunk_tasks(2)
        elif c == 1:
            tk += q_chunk_tasks(3) + q_chunk_tasks(4)
        elif c == 2:
            tk += q_chunk_tasks(5) + q_chunk_tasks(6)
        elif c == 3:
            tk += q_chunk_tasks(7)
        tk += k_chunk_tasks(c)
        if c == 0:
            for tc_i in range(4):
                tk += v_chunk_tasks(tc_i, 0)
        elif c == 4:
            for tc_i in range(4):
                tk += v_chunk_tasks(tc_i, 1)
        return tk

    pend = None  # (g, vp_t, pe, po) one stage behind
    kp_next = vp_next = None
    for c in range(NC_CHUNKS):
        tasks = deque(build_tasks(c))
        # vp new-block copies must follow v-proj tasks; they feed wv g>=8
        tasks.extend(vp_new_tasks(c, vp_cur_t))
        if c == 4:
            def value_out():
                nc.sync.dma_start(
                    D["value"].ap().rearrange("(tc p) o -> p tc o", p=128),
                    vnew_t[:].rearrange("p (tc o) -> p tc o", tc=4))
            tasks.append(value_out)
        budget = (len(tasks) + 9) // 10
        for g in range(10):
            for _ in range(budget):
                if tasks:
                    tasks.popleft()()
            if g == 1 and c > 0:
                emit_norm(c - 1)
            pe_t, po_t = emit_scores(c, g, kp_cur)
            if pend is not None:
                emit_wv(*pend)
            pend = (g, vp_cur_t, pe_t, po_t)
            # prefetch after the delayed wv so the kv ring's WAR deps see
            # every reader of the buffer being recycled
            if g == 0 and c < NC_CHUNKS - 1:
                kp_next, vp_next = fetch_kv(c + 1)
        while tasks:
            tasks.popleft()()
        kp_cur, vp_cur_t = kp_next, vp_next

    emit_wv(*pend)
    emit_norm(NC_CHUNKS - 1)

    # ---------------- output projection + output DMAs ----------------
    nc.sync.dma_start(
        D["keyT"].ap().rearrange("(m p) t -> p m t", p=128),
        knTf_t[:].rearrange("p (m t) -> p m t", m=8))
    for m in range(8):
        po = pmisc.tile([128, 512], F32, name="po", tag="pp")
        for cc in range(8):
            nc.tensor.matmul(po[:], lhsT=wo_t[cc][:, m * 128:(m + 1) * 128],
                             rhs=wv_tiles[cc][:], start=(cc == 0),
                             stop=(cc == 7))
        nc.vector.tensor_scalar(
            outT_t[:, m * 512:(m + 1) * 512], po[:], 1.0, bop_t[:, m:m + 1],
            ALU.mult, ALU.add)
    nc.sync.dma_start(
        D["outT"].ap().rearrange("(m p) t -> p m t", p=128),
        outT_t[:].rearrange("p (m t) -> p m t", m=8))


def build():
    nc = bacc.Bacc("TRN2", target_bir_lowering=False, debug=False)
    D = {}
    D["xT"] = nc.dram_tensor("xT", [NS, T], F16, kind="ExternalInput")
    D["kTc"] = nc.dram_tensor("kTc", [NS, S], F16, kind="ExternalInput")
    D["vaug"] = nc.dram_tensor("vaug", [128, NC_CHUNKS * 2080], F16,
                               kind="ExternalInput")
    for w in ("Wq", "Wk", "Wv", "Wo"):
        D[w] = nc.dram_tensor(w, [NS, NS], F16, kind="ExternalInput")
    D["bqs"] = nc.dram_tensor("bqs", [128, 8], F32, kind="ExternalInput")
    D["bop"] = nc.dram_tensor("bop", [128, 8], F32, kind="ExternalInput")
    D["bv"] = nc.dram_tensor("bv", [1, NS], F16, kind="ExternalInput")
    D["ones"] = nc.dram_tensor("ones", [128, 136], F16, kind="ExternalInput")
    D["outT"] = nc.dram_tensor("outT", [NS, T], F32, kind="ExternalOutput")
    D["keyT"] = nc.dram_tensor("keyT", [NS, T], F32, kind="ExternalOutput")
    D["value"] = nc.dram_tensor("value", [T, NS], F32, kind="ExternalOutput")

    with tile.TileContext(nc) as tc:
        with ExitStack() as ctx:
            _emit(ctx, tc, D)
    nc.compile()
    return nc


_NC_CACHE = None


def _get_nc():
    global _NC_CACHE
    if _NC_CACHE is None:
        _NC_CACHE = build()
    return _NC_CACHE


def kernel(x, kv_cache, offset=0, Wq=None, bq=None, Wk=None, Wv=None, bv=None,
           Wo=None, bo=None, trace=False):
    global LAST_EXEC_NS, LAST_RESULTS
    x = np.asarray(x, np.float32)
    kv_cache = np.asarray(kv_cache, np.float32)
    Wq, bq, Wk, Wv, bv, Wo, bo = [
        np.asarray(a, np.float32) for a in (Wq, bq, Wk, Wv, bv, Wo, bo)]

    # shared (batch-independent) prep, fp16 weights
    shared = {
        "Wq": Wq.astype(np.float16), "Wk": Wk.astype(np.float16),
        "Wv": Wv.astype(np.float16), "Wo": Wo.astype(np.float16),
        "bqs": np.ascontiguousarray((bq * SCALE2).reshape(8, 128).T,
                                    dtype=np.float32),
        "bop": np.ascontiguousarray(bo.reshape(8, 128).T, dtype=np.float32),
        "bv": bv[None, :].astype(np.float16),
        "ones": np.ones((128, 136), np.float16),
    }

    xT16 = np.ascontiguousarray(x.transpose(0, 2, 1)).astype(np.float16)
    kT16 = np.ascontiguousarray(
        kv_cache[:, 0, 0].transpose(0, 2, 1)).astype(np.float16)
    # vaug[b, p, c*2080 + j*130 + e*65 + q]: per head-pair c, s-chunk j,
    # parity e: [64 v dims | 1.0]
    vc = kv_cache[:, 0, 1]                                  # [B, S, NS]
    v6 = vc.reshape(B, 16, 128, 8, 2, 64).transpose(0, 2, 3, 1, 4, 5)
    vaug = np.ones((B, 128, 8, 16, 2, 65), np.float16)
    vaug[..., 0:64] = v6
    vaug = vaug.reshape(B, 128, NC_CHUNKS * 2080)

    in_maps = []
    for b in range(B):
        m = dict(shared)
        m["xT"] = xT16[b]
        m["kTc"] = kT16[b]
        m["vaug"] = vaug[b]
        in_maps.append(m)

    nc = _get_nc()
    res = run_bass_kernel_spmd(nc, in_maps, core_ids=list(range(B)),
                               trace=trace)
    LAST_EXEC_NS = res.exec_time_ns
    LAST_RESULTS = res
    out = np.stack([res.results[b]["outT"].T for b in range(B)])
    key = np.stack([res.results[b]["keyT"].T for b in range(B)])
    value = np.stack([res.results[b]["value"] for b in range(B)])
    return (np.ascontiguousarray(out), np.ascontiguousarray(key),
            np.ascontiguousarray(value))


# revision 13
# speedup vs baseline: 1.7708x; 1.2855x over previous
"""Trainium2 Bass kernel: cached multi-head self-attention decoder block.

Per-core (batch-parallel, B=8 -> 8 cores) computation for batch b, fp16
matmul operands (fp16 streams the PE moving operand at 1 elem/cycle vs
2 cyc/elem for fp32), f32 PSUM accumulation, f32 outputs:

  q  = x @ Wq + bq        (qT [NS, T] fp16, pre-scaled by HD^-0.5)
  kn = x @ Wk             (knT fp16 for scores + f32 copy for keyT out)
  vn = x @ Wv + bv        (vnew f32 [T, NS] for value out + fp16 vp blocks)
  scoresT[h] = kh^T-slices x qh -> [S+T, T] per head (s on partitions)
  probsT = exp(scoresT - 8)  (fp16; bias keeps probs <= 1 in fp16 range)
  o[h]   = [vh | 1]^T @ probsT accumulated over s-chunks in PSUM
           rows 0..63 = unnormalized o^T, row 64 = softmax denominator
  wvT    = o * (1/denom broadcast via K=1 matmul ones x recip row)
  outT   = Wo^T @ wvT + bo

The attention phase is bound by the ScalarE exp stream (~178us); all
projection matmuls are interleaved into it as filler PE work via a task
queue, and the per-g wv matmuls are software-pipelined one stage behind
the score matmuls so the PE never stalls on the ACT->probs dependency.
"""

import numpy as np
from collections import deque
from contextlib import ExitStack

import concourse.bass as bass
import concourse.tile as tile
from concourse import bacc, mybir
from concourse.bass_utils import run_bass_kernel_spmd

F32 = mybir.dt.float32
F32R = mybir.dt.float32r
F16 = mybir.dt.float16
ALU = mybir.AluOpType
ACTF = mybir.ActivationFunctionType

B, T, S, NS, NH, HD = 8, 512, 2048, 1024, 16, 64
ST = S + T             # 2560
NC_CHUNKS = NS // 128  # 8 head pairs
SCN = ST // 128        # 20 s chunks (16 cache + 4 new)
SCALE2 = float(HD ** -0.5)  # fold both q and k scales into q
EXP_BIAS = -8.0        # probs = exp(scores - 8) <= ~1 : fp16-safe

LAST_EXEC_NS = None
LAST_RESULTS = None


def _emit(ctx, tc, D):
    nc = tc.nc

    # ---------------- pools ----------------
    const = ctx.enter_context(tc.tile_pool(name="const", bufs=1))
    wpool = ctx.enter_context(tc.tile_pool(name="weights", bufs=1))
    pers = ctx.enter_context(tc.tile_pool(name="pers", bufs=1))
    wvp = ctx.enter_context(tc.tile_pool(name="wvp", bufs=1))
    kpool = ctx.enter_context(tc.tile_pool(name="kpair", bufs=2))
    vpool = ctx.enter_context(tc.tile_pool(name="vpair", bufs=2))
    probs = ctx.enter_context(tc.tile_pool(name="probs", bufs=4))
    rpool = ctx.enter_context(tc.tile_pool(name="rtiles", bufs=2))
    spool = ctx.enter_context(tc.tile_pool(name="spsum", bufs=2, space="PSUM"))
    pvpool = ctx.enter_context(tc.tile_pool(name="pvpsum", bufs=1, space="PSUM"))
    pmisc = ctx.enter_context(tc.tile_pool(name="pmisc", bufs=2, space="PSUM"))

    # ---------------- constants (xT first: q-proj gates the pipeline) ----
    xT_t = const.tile([128, 4096], F16, name="xT_t")  # k-chunk-major cols
    nc.gpsimd.dma_start(
        xT_t[:].rearrange("p (k t) -> p k t", k=8),
        D["xT"].ap().rearrange("(k p) t -> p k t", p=128),
    )
    bqs_t = const.tile([128, 8], F32, name="bqs_t")
    nc.gpsimd.dma_start(bqs_t[:], D["bqs"][:, :])
    ebias_t = const.tile([128, 1], F32, name="ebias_t")
    nc.gpsimd.dma_start(ebias_t[:], D["ebias"][:, :])
    bop_t = const.tile([128, 8], F32, name="bop_t")
    nc.gpsimd.dma_start(bop_t[:], D["bop"][:, :])
    bv_t = const.tile([1, NS], F16, name="bv_t")
    nc.gpsimd.dma_start(bv_t[:], D["bv"][:, :])
    ones1 = const.tile([1, 128], F16, name="ones1")
    nc.gpsimd.dma_start(ones1[:], D["ones"].ap()[0:1, 0:128])
    ones64f = const.tile([128, 64], F32R, name="ones64f")
    nc.gpsimd.dma_start(ones64f[:], D["onesf"][:, :])

    # ---------------- weights: full fp16 preload on sync queue ----------------
    # DMA order = need order: Wq (q proj prologue), Wv (v-proj from c=0),
    # Wk (k-proj m=0 by c=0 g6), Wo (tail).
    wq_t = [wpool.tile([128, 1024], F16, name=f"wq{k}", tag=f"wq{k}")
            for k in range(8)]
    wv_t = [wpool.tile([128, 1024], F16, name=f"wvw{k}", tag=f"wvw{k}")
            for k in range(8)]
    wk_t = [wpool.tile([128, 1024], F16, name=f"wk{k}", tag=f"wk{k}")
            for k in range(8)]
    wo_t = [wpool.tile([128, 1024], F16, name=f"wo{k}", tag=f"wo{k}")
            for k in range(8)]
    for k in range(8):
        nc.sync.dma_start(wq_t[k][:], D["Wq"].ap()[k * 128:(k + 1) * 128, :])
    for k in range(8):
        nc.sync.dma_start(wv_t[k][:], D["Wv"].ap()[k * 128:(k + 1) * 128, :])
    for k in range(8):
        nc.sync.dma_start(wk_t[k][:], D["Wk"].ap()[k * 128:(k + 1) * 128, :])
    for k in range(8):
        nc.sync.dma_start(wo_t[k][:], D["Wo"].ap()[k * 128:(k + 1) * 128, :])

    # ---------------- persistent activations ----------------
    qT_t = pers.tile([128, 4096], F16, name="qT_t")    # m-chunk-major
    knT_t = pers.tile([128, 4096], F16, name="knT_t")  # m-chunk-major
    knTf_t = pers.tile([128, 4096], F32, name="knTf_t")  # exact f32 for keyT
    vnew_t = pers.tile([128, 4096], F32, name="vnew_t")  # tc-major [t, ns]
    outT_t = pers.tile([128, 4096], F32, name="outT_t")
    wv_tiles = [wvp.tile([128, 512], F16, name=f"wv_{c}", tag=f"wv{c}")
                for c in range(NC_CHUNKS)]

    # ---------------- projection task emitters ----------------
    def q_chunk_tasks(m):
        st = {}
        out = []
        for k in range(8):
            def tk(k=k, m=m):
                if k == 0:
                    st["p"] = pmisc.tile([128, 512], F32, name=f"pq{m}", tag="pp")
                nc.tensor.matmul(
                    st["p"][:], lhsT=wq_t[k][:, m * 128:(m + 1) * 128],
                    rhs=xT_t[:, k * 512:(k + 1) * 512],
                    start=(k == 0), stop=(k == 7))
            out.append(tk)

        def tf(m=m):
            nc.vector.tensor_scalar(
                qT_t[:, m * 512:(m + 1) * 512], st["p"][:], SCALE2,
                bqs_t[:, m:m + 1], ALU.mult, ALU.add)
        out.append(tf)
        return out

    def k_chunk_tasks(m):
        st = {}
        out = []
        for k in range(8):
            def tk(k=k, m=m):
                if k == 0:
                    st["p"] = pmisc.tile([128, 512], F32, name=f"pk{m}", tag="pp")
                nc.tensor.matmul(
                    st["p"][:], lhsT=wk_t[k][:, m * 128:(m + 1) * 128],
                    rhs=xT_t[:, k * 512:(k + 1) * 512],
                    start=(k == 0), stop=(k == 7))
            out.append(tk)

        def tf(m=m):
            nc.vector.tensor_copy(knT_t[:, m * 512:(m + 1) * 512], st["p"][:])
            nc.vector.tensor_copy(knTf_t[:, m * 512:(m + 1) * 512], st["p"][:])
        out.append(tf)
        return out

    def v_chunk_tasks(tc_i, oh):
        st = {}
        out = []
        for k in range(8):
            def tk(k=k, tc_i=tc_i, oh=oh):
                if k == 0:
                    st["p"] = pmisc.tile([128, 512], F32,
                                         name=f"pv{tc_i}{oh}", tag="pp")
                nc.tensor.matmul(
                    st["p"][:],
                    lhsT=xT_t[:, k * 512 + tc_i * 128:k * 512 + (tc_i + 1) * 128],
                    rhs=wv_t[k][:, oh * 512:(oh + 1) * 512],
                    start=(k == 0), stop=False)
            out.append(tk)

        def tb(tc_i=tc_i, oh=oh):
            nc.tensor.matmul(
                st["p"][:], lhsT=ones1[:],
                rhs=bv_t[0:1, oh * 512:(oh + 1) * 512],
                start=False, stop=True)
            nc.vector.tensor_copy(
                vnew_t[:, tc_i * 1024 + oh * 512:tc_i * 1024 + (oh + 1) * 512],
                st["p"][:])
        out.append(tb)
        return out

    def vp_new_tasks(c, vp_t):
        out = []
        for tc_i in range(4):
            def tcpy(tc_i=tc_i, c=c, vp_t=vp_t):
                nc.vector.tensor_copy(
                    vp_t[:, 2080 + tc_i * 130:2080 + (tc_i + 1) * 130]
                    .rearrange("p (h q) -> p h q", q=65)[:, :, 0:64],
                    vnew_t[:, tc_i * 1024 + c * 128:tc_i * 1024 + (c + 1) * 128]
                    .rearrange("p (h q) -> p h q", q=64))
            out.append(tcpy)
        return out

    # ---------------- attention primitives ----------------
    def fetch_kv(c):
        kp = kpool.tile([128, 2048], F16, name="kp", tag="kp")
        nc.gpsimd.dma_start(kp[:], D["kTc"].ap()[c * 128:(c + 1) * 128, :])
        vp = vpool.tile([128, 2600], F16, name="vp", tag="vp")
        nc.gpsimd.dma_start(vp[:, 0:2080],
                            D["vaug"].ap()[:, c * 2080:(c + 1) * 2080])
        nc.gpsimd.dma_start(
            vp[:, 2080:2600].rearrange("p (tc h q) -> p tc h q", h=2, q=65)
            [:, :, :, 64:65],
            D["ones"].ap()[:, 128:136].rearrange(
                "p (tc h) -> p tc h", h=2)[:, :, :, None])
        return kp, vp

    def emit_scores(c, g, kp_t):
        se = spool.tile([128, 1024], F32, name="se", tag="sc")
        so = spool.tile([128, 1024], F32, name="so", tag="sc")
        rhs_e = qT_t[0:64, c * 512:(c + 1) * 512]
        rhs_o = qT_t[64:128, c * 512:(c + 1) * 512]
        for jj in range(2):
            j = 2 * g + jj
            if j < 16:
                le = kp_t[0:64, j * 128:(j + 1) * 128]
                lo = kp_t[64:128, j * 128:(j + 1) * 128]
            else:
                jo = c * 512 + (j - 16) * 128
                le = knT_t[0:64, jo:jo + 128]
                lo = knT_t[64:128, jo:jo + 128]
            nc.tensor.matmul(se[:, jj * 512:(jj + 1) * 512], lhsT=le,
                             rhs=rhs_e, start=True, stop=True)
            nc.tensor.matmul(so[:, jj * 512:(jj + 1) * 512], lhsT=lo,
                             rhs=rhs_o, start=True, stop=True)
        pe_t = probs.tile([128, 1024], F16, name="pe_t", tag="pr")
        nc.scalar.activation(pe_t[:], se[:], ACTF.Exp, bias=ebias_t[:])
        po_t = probs.tile([128, 1024], F16, name="po_t", tag="pr")
        nc.scalar.activation(po_t[:], so[:], ACTF.Exp, bias=ebias_t[:])
        return pe_t, po_t

    pv_cur = {}

    def emit_wv(g, vp_t, pe_t, po_t):
        if g == 0:
            pv_cur["e"] = pvpool.tile([65, 512], F32, name="pve", tag="pv_e")
            pv_cur["o"] = pvpool.tile([65, 512], F32, name="pvo", tag="pv_o")
        pve, pvo = pv_cur["e"], pv_cur["o"]
        for jj in range(2):
            j = 2 * g + jj
            nc.tensor.matmul(pve[:], lhsT=vp_t[:, j * 130:j * 130 + 65],
                             rhs=pe_t[:, jj * 512:(jj + 1) * 512],
                             start=(j == 0), stop=(j == SCN - 1))
            nc.tensor.matmul(pvo[:], lhsT=vp_t[:, j * 130 + 65:(j + 1) * 130],
                             rhs=po_t[:, jj * 512:(jj + 1) * 512],
                             start=(j == 0), stop=(j == SCN - 1))

    def emit_norm(c):
        # Stage PSUM accumulators to SBUF immediately (frees the pv banks
        # for the next chunk), then broadcast the denominator row via a K=1
        # matmul and take a fast approximate reciprocal on 64 partitions.
        pve, pvo = pv_cur.pop("e"), pv_cur.pop("o")
        sve = rpool.tile([65, 512], F32R, name="sve", tag="sv")
        nc.vector.tensor_copy(sve[:], pve[:])
        svo = rpool.tile([65, 512], F32R, name="svo", tag="sv")
        nc.vector.tensor_copy(svo[:], pvo[:])
        with nc.allow_low_precision(reason="1/denom scale, ~18 bits"):
            rbe = pmisc.tile([128, 512], F32, name="rbe", tag="pp")
            nc.tensor.matmul(rbe[0:64, :], lhsT=ones64f[64:65, 0:64],
                             rhs=sve[64:65, :], start=True, stop=True)
            rbse = rpool.tile([64, 512], F32, name="rbse", tag="rb")
            nc.vector.reciprocal_approx_fast(rbse[:], rbe[0:64, :])
            nc.vector.tensor_mul(wv_tiles[c][0:64, :], sve[0:64, :], rbse[:])
            rbo = pmisc.tile([128, 512], F32, name="rbo", tag="pp")
            nc.tensor.matmul(rbo[0:64, :], lhsT=ones64f[64:65, 0:64],
                             rhs=svo[64:65, :], start=True, stop=True)
            rbso = rpool.tile([64, 512], F32, name="rbso", tag="rb")
            nc.vector.reciprocal_approx_fast(rbso[:], rbo[0:64, :])
            tmo = rpool.tile([64, 512], F16, name="tmo", tag="tm")
            nc.vector.tensor_mul(tmo[:], svo[0:64, :], rbso[:])
        # partition shift 0..63 -> 64..127 via SBUF->SBUF DMA
        nc.sync.dma_start(wv_tiles[c][64:128, :], tmo[:])

    # ---------------- prologue ----------------
    kp_cur, vp_cur_t = fetch_kv(0)
    for t in q_chunk_tasks(0):
        t()
    for t in q_chunk_tasks(1):
        t()

    # ---------------- main attention loop with interleaved projections ----
    # Per-c task lists balance PE load: c=0 must carry all of v-proj oh=0
    # (feeds c=0's own new-token wv), oh=1 spreads over c=1..3, k-proj m=c
    # feeds c's own g>=8 scores, q-proj m feeds chunk m's scores (m>=c+2).
    def build_tasks(c):
        tk = []
        if c == 0:
            tk += q_chunk_tasks(2)
            tk += k_chunk_tasks(0)
            for tc_i in range(4):
                tk += v_chunk_tasks(tc_i, 0)
        elif c == 1:
            tk += q_chunk_tasks(3) + q_chunk_tasks(4)
            tk += k_chunk_tasks(1)
            tk += v_chunk_tasks(0, 1)
        elif c == 2:
            tk += q_chunk_tasks(5) + q_chunk_tasks(6)
            tk += k_chunk_tasks(2)
            tk += v_chunk_tasks(1, 1)
        elif c == 3:
            tk += q_chunk_tasks(7)
            tk += k_chunk_tasks(3)
            tk += v_chunk_tasks(2, 1) + v_chunk_tasks(3, 1)
        else:
            tk += k_chunk_tasks(c)
        return tk

    pend = None  # (g, vp_t, pe, po) one stage behind
    kp_next = vp_next = None
    for c in range(NC_CHUNKS):
        tasks = deque(build_tasks(c))
        # vp new-block copies must follow v-proj tasks; they feed wv g>=8
        tasks.extend(vp_new_tasks(c, vp_cur_t))
        if c == 4:
            def value_out():
                nc.sync.dma_start(
                    D["value"].ap().rearrange("(tc p) o -> p tc o", p=128),
                    vnew_t[:].rearrange("p (tc o) -> p tc o", tc=4))
            tasks.append(value_out)
        budget = (len(tasks) + 9) // 10
        for g in range(10):
            if g == 1 and c > 0:
                emit_norm(c - 1)
            pe_t, po_t = emit_scores(c, g, kp_cur)
            if pend is not None:
                emit_wv(*pend)
            pend = (g, vp_cur_t, pe_t, po_t)
            # prefetch after the delayed wv so the kv ring's WAR deps see
            # every reader of the buffer being recycled
            if g == 0 and c < NC_CHUNKS - 1:
                kp_next, vp_next = fetch_kv(c + 1)
            for _ in range(budget):
                if tasks:
                    tasks.popleft()()
        while tasks:
            tasks.popleft()()
        kp_cur, vp_cur_t = kp_next, vp_next

    emit_wv(*pend)
    emit_norm(NC_CHUNKS - 1)

    # ---------------- output projection + output DMAs ----------------
    nc.sync.dma_start(
        D["keyT"].ap().rearrange("(m p) t -> p m t", p=128),
        knTf_t[:].rearrange("p (m t) -> p m t", m=8))
    for m in range(8):
        po = pmisc.tile([128, 512], F32, name="po", tag="pp")
        for cc in range(8):
            nc.tensor.matmul(po[:], lhsT=wo_t[cc][:, m * 128:(m + 1) * 128],
                             rhs=wv_tiles[cc][:], start=(cc == 0),
                             stop=(cc == 7))
        nc.vector.tensor_scalar(
            outT_t[:, m * 512:(m + 1) * 512], po[:], 1.0, bop_t[:, m:m + 1],
            ALU.mult, ALU.add)
        # stream each outT m-chunk out as soon as it is ready
        nc.sync.dma_start(D["outT"].ap()[m * 128:(m + 1) * 128, :],
                          outT_t[:, m * 512:(m + 1) * 512])


def build():
    nc = bacc.Bacc("TRN2", target_bir_lowering=False, debug=False)
    D = {}
    D["xT"] = nc.dram_tensor("xT", [NS, T], F16, kind="ExternalInput")
    D["kTc"] = nc.dram_tensor("kTc", [NS, S], F16, kind="ExternalInput")
    D["vaug"] = nc.dram_tensor("vaug", [128, NC_CHUNKS * 2080], F16,
                               kind="ExternalInput")
    for w in ("Wq", "Wk", "Wv", "Wo"):
        D[w] = nc.dram_tensor(w, [NS, NS], F16, kind="ExternalInput")
    D["bqs"] = nc.dram_tensor("bqs", [128, 8], F32, kind="ExternalInput")
    D["bop"] = nc.dram_tensor("bop", [128, 8], F32, kind="ExternalInput")
    D["bv"] = nc.dram_tensor("bv", [1, NS], F16, kind="ExternalInput")
    D["ones"] = nc.dram_tensor("ones", [128, 136], F16, kind="ExternalInput")
    D["onesf"] = nc.dram_tensor("onesf", [128, 64], F32, kind="ExternalInput")
    D["ebias"] = nc.dram_tensor("ebias", [128, 1], F32, kind="ExternalInput")
    D["outT"] = nc.dram_tensor("outT", [NS, T], F32, kind="ExternalOutput")
    D["keyT"] = nc.dram_tensor("keyT", [NS, T], F32, kind="ExternalOutput")
    D["value"] = nc.dram_tensor("value", [T, NS], F32, kind="ExternalOutput")

    with tile.TileContext(nc) as tc:
        with ExitStack() as ctx:
            _emit(ctx, tc, D)
    nc.compile()
    return nc


_NC_CACHE = None


def _get_nc():
    global _NC_CACHE
    if _NC_CACHE is None:
        _NC_CACHE = build()
    return _NC_CACHE


def kernel(x, kv_cache, offset=0, Wq=None, bq=None, Wk=None, Wv=None, bv=None,
           Wo=None, bo=None, trace=False):
    global LAST_EXEC_NS, LAST_RESULTS
    x = np.asarray(x, np.float32)
    kv_cache = np.asarray(kv_cache, np.float32)
    Wq, bq, Wk, Wv, bv, Wo, bo = [
        np.asarray(a, np.float32) for a in (Wq, bq, Wk, Wv, bv, Wo, bo)]

    # shared (batch-independent) prep, fp16 weights
    shared = {
        "Wq": Wq.astype(np.float16), "Wk": Wk.astype(np.float16),
        "Wv": Wv.astype(np.float16), "Wo": Wo.astype(np.float16),
        "bqs": np.ascontiguousarray((bq * SCALE2).reshape(8, 128).T,
                                    dtype=np.float32),
        "bop": np.ascontiguousarray(bo.reshape(8, 128).T, dtype=np.float32),
        "bv": bv[None, :].astype(np.float16),
        "ones": np.ones((128, 136), np.float16),
        "onesf": np.ones((128, 64), np.float32),
        "ebias": np.full((128, 1), EXP_BIAS, np.float32),
    }

    xT16 = np.ascontiguousarray(x.transpose(0, 2, 1)).astype(np.float16)
    kT16 = np.ascontiguousarray(
        kv_cache[:, 0, 0].transpose(0, 2, 1)).astype(np.float16)
    # vaug[b, p, c*2080 + j*130 + e*65 + q]: per head-pair c, s-chunk j,
    # parity e: [64 v dims | 1.0]
    vc = kv_cache[:, 0, 1]                                  # [B, S, NS]
    v6 = vc.reshape(B, 16, 128, 8, 2, 64).transpose(0, 2, 3, 1, 4, 5)
    vaug = np.ones((B, 128, 8, 16, 2, 65), np.float16)
    vaug[..., 0:64] = v6
    vaug = vaug.reshape(B, 128, NC_CHUNKS * 2080)

    in_maps = []
    for b in range(B):
        m = dict(shared)
        m["xT"] = xT16[b]
        m["kTc"] = kT16[b]
        m["vaug"] = vaug[b]
        in_maps.append(m)

    nc = _get_nc()
    res = run_bass_kernel_spmd(nc, in_maps, core_ids=list(range(B)),
                               trace=trace)
    LAST_EXEC_NS = res.exec_time_ns
    LAST_RESULTS = res
    out = np.stack([res.results[b]["outT"].T for b in range(B)])
    key = np.stack([res.results[b]["keyT"].T for b in range(B)])
    value = np.stack([res.results[b]["value"] for b in range(B)])
    return (np.ascontiguousarray(out), np.ascontiguousarray(key),
            np.ascontiguousarray(value))
